# revision 26
# baseline (speedup 1.0000x reference)
"""NNConv (gnn_message_passing) Bass kernel for 8 Trainium2 NeuronCores.

Strategy (edge-parallel, dst-sharded):
- Host relabels nodes with a permutation so that the 16384 nodes form 128
  "windows" of 128 nodes, each window receiving exactly U edges (by
  destination).  Core c owns windows [16c, 16c+16): 2048 nodes / 8192 edges.
  Pure SPMD; per-core variation lives only in the input data.
- Per 128-edge tile, on device (all matmuls bf16/fp8):
    P   = attr_aug^T @ Aaug            (PE matmul -> PSUM f32, K=3)
    q   = relu(P) * x[src]             (evacuation, split across engines)
    agg += onehot(dst)^T @ q           (PE matmul; zero-step output AP sums
                                        the c_in axis while contracting edges)
- PSUM evacuation is the bottleneck (DVE/ScalarE are the only engines that
  can read PSUM).  Per-unit modes balance the two:
    dr : DVE scalar_tensor_tensor PSUM->fp8 (i,o); scatter = fp8 DoubleRow
         matmuls over tile PAIRS (halved PE cost).
    se : ScalarE Relu activation PSUM->SBUF bf16 in (o,i) order, then DVE
         tensor_tensor bf16 at 2x rate (all operands 2-byte, inner step 1),
         scatter = bf16 matmuls with (i outer, o inner) strided rhs.
- h1 is exchanged between layers with AllGathers into a Shared scratchpad;
  a dummy 1-row AllGather at program start hoists the one-time collective
  stream init/barrier into the DMA prologue.
"""

import numpy as np
import ml_dtypes
from contextlib import ExitStack

import concourse.bass as bass
import concourse.tile as tile
from concourse import bacc, mybir
from concourse.bass import IndirectOffsetOnAxis
from concourse.bass_utils import run_bass_kernel_spmd

dt = mybir.dt
BF16 = ml_dtypes.bfloat16
FP8 = ml_dtypes.float8_e4m3

N = 16384
E = 65536
NCORES = 8
P = 128                 # partitions / edges per tile
WINDOWS = 128           # global 128-node windows
WPC = WINDOWS // NCORES  # 16 windows per core
NPC = N // NCORES        # 2048 nodes per core
COUT = 64
CIN1 = 8
CIN2 = 64
# AllGather chunk boundaries in windows-per-core.  A Shared-output
# AllGather must be a single instruction (single-writer rule), but the
# shared-scratchpad path only writes each core's own slice, so one big
# AG at the layer boundary is cheap.
AGB = [0, 16]
# L2 unit modes, all (i,o)-ordered:
#   dr : DVE scalar_tensor_tensor PSUM->fp8 pair buffer, DoubleRow scatter
#   sv2: ScalarE relu PSUM->SBUF bf16, then DVE tensor_tensor at 2x rate
#        (x duplicated into adjacent pairs so every operand has a step-1
#        innermost dim), bf16 contiguous scatter
# dr-units per tile-pair cycles 1,1,1,2 to balance DVE vs ScalarE; the
# first PREWORK_PAIRS pairs use 0 so the whole tile evacuates through
# ScalarE while the h1 AllGather is still in flight.
DR_CYCLE = [1, 1, 1, 2]
PREWORK_PAIRS = 4
SCAT_LAG = 3    # L2: pend entries (tiles); L1 uses pairs
L1_LAG = 2
GPRE = 6        # gather prefetch distance (tiles)
WARM_MM = 16    # prologue matmuls that warm the PE HAM clock gate

_cached = {}


def _build_program(U):
    """Build the SPMD Bass program. U = edges per window (multiple of 128)."""
    T = U // P    # tiles per window
    NT = WPC * T  # tiles per core per layer
    EPC = WPC * U

    nc = bacc.Bacc("TRN2", target_bir_lowering=False, debug=False,
                   num_devices=NCORES)

    # attr and A replicated at partition quadrants 0/32/64/96 so four K=3
    # generator matmuls can run concurrently in distinct PE row groups.
    attrT_d = nc.dram_tensor("attrT", [99, EPC], dt.bfloat16, kind="ExternalInput").ap()
    srcw_d = nc.dram_tensor("srcw", [P, NT], dt.int32, kind="ExternalInput").ap()
    A1_d = nc.dram_tensor("A1aug", [99, CIN1 * COUT], dt.bfloat16, kind="ExternalInput").ap()
    A2_d = nc.dram_tensor("A2aug", [99, CIN2 * COUT], dt.bfloat16, kind="ExternalInput").ap()
    # host-precomputed layer-1 gathered features and per-tile one-hot
    # matrices (bf16 for bf16 scatters, fp8 for the DoubleRow scatters)
    xsg_d = nc.dram_tensor("xsg", [P, NT * CIN1], dt.bfloat16, kind="ExternalInput").ap()
    xsg2_d = nc.dram_tensor("xsg2", [P, NT * 2 * CIN1], dt.bfloat16, kind="ExternalInput").ap()
    oh1_d = nc.dram_tensor("oh1", [P, NT * P], dt.bfloat16, kind="ExternalInput").ap()
    oh2_d = nc.dram_tensor("oh2", [P, NT * P], dt.float8e4, kind="ExternalInput").ap()
    xT_d = nc.dram_tensor("xT9", [CIN1 + 1, NPC], dt.bfloat16, kind="ExternalInput").ap()
    r1_d = nc.dram_tensor("r1aug", [CIN1 + 1, COUT], dt.bfloat16, kind="ExternalInput").ap()
    r2_d = nc.dram_tensor("r2aug", [CIN2 + 1, COUT], dt.bfloat16, kind="ExternalInput").ap()
    iota_d = nc.dram_tensor("iota", [P, P], dt.bfloat16, kind="ExternalInput").ap()
    out_d = nc.dram_tensor("out", [NPC, COUT], dt.float32, kind="ExternalOutput").ap()

    with tile.TileContext(nc) as tc, ExitStack() as ctx:
        consts = ctx.enter_context(tc.tile_pool(name="consts", bufs=1))
        xgp = ctx.enter_context(tc.tile_pool(name="xgp", bufs=8))
        ohp = ctx.enter_context(tc.tile_pool(name="ohp", bufs=6))
        qp = ctx.enter_context(tc.tile_pool(name="qp", bufs=10))
        rup = ctx.enter_context(tc.tile_pool(name="rup", bufs=8))
        outp = ctx.enter_context(tc.tile_pool(name="outp", bufs=3))
        pp = ctx.enter_context(tc.tile_pool(name="pp", bufs=3, space="PSUM"))
        aggp = ctx.enter_context(tc.tile_pool(name="aggp", bufs=2, space="PSUM"))
        dramp = ctx.enter_context(tc.tile_pool(name="dram", bufs=1, space="DRAM"))

        # h1 slice (local) and allgathered h1 (global, shared scratchpad)
        hloc = dramp.tile([NPC, COUT], dt.bfloat16)
        hglob = dramp.tile([N, COUT], dt.bfloat16, addr_space="Shared")
        # dummy 1-row AllGather issued first: pulls the one-time collective
        # stream init + device barrier into the DMA/compute prologue so the
        # real h1 exchange isn't serialized behind it.  It reads a scratch
        # tile (not hloc) so no write-after-read dep blocks layer 1.
        dumi = dramp.tile([1, COUT], dt.bfloat16)
        dumg = dramp.tile([NCORES, COUT], dt.bfloat16, addr_space="Shared")
        nc.gpsimd.collective_compute(
            "AllGather", mybir.AluOpType.bypass,
            replica_groups=[list(range(NCORES))],
            ins=[dumi[:].opt()],
            outs=[dumg[:].opt()])

        # --- constant loads, spread across engine queues so the ~0.8us
        # per-dma_start dispatch cost doesn't serialize the prologue ---
        A1_s = consts.tile([99, CIN1 * COUT], dt.bfloat16)
        A2_s = consts.tile([99, CIN2 * COUT], dt.bfloat16)
        srcw_s = consts.tile([P, NT], dt.int32)
        xT9_s = consts.tile([CIN1 + 1, NPC], dt.bfloat16)
        iota_s = consts.tile([P, P], dt.bfloat16)
        r1_s = consts.tile([CIN1 + 1, COUT], dt.bfloat16)
        r2_s = consts.tile([CIN2 + 1, COUT], dt.bfloat16)
        attr_s = consts.tile([99, EPC], dt.bfloat16)
        xsg_s = consts.tile([P, NT * CIN1], dt.bfloat16)
        xsg2_s = consts.tile([P, NT * 2 * CIN1], dt.bfloat16)
        oh1_s = consts.tile([P, NT * P], dt.bfloat16)
        oh2_s = consts.tile([P, NT * P], dt.float8e4)

        # L1-critical small tensors first on each queue
        nc.sync.dma_start(iota_s[:], iota_d[:])
        nc.sync.dma_start(r1_s[:], r1_d[:])
        nc.scalar.dma_start(A1_s[:], A1_d[:])
        nc.scalar.dma_start(xT9_s[:], xT_d[:])
        nc.gpsimd.dma_start(xsg_s[:], xsg_d[:])
        nc.gpsimd.dma_start(xsg2_s[:], xsg2_d[:])
        nc.gpsimd.dma_start(srcw_s[:], srcw_d[:])
        # big tensors in column chunks, round-robined across three queues;
        # early chunks (first windows) land first
        engs = [nc.sync, nc.scalar, nc.gpsimd]
        big = [(attr_s, attrT_d, 4), (oh1_s, oh1_d, 4), (oh2_s, oh2_d, 2)]
        for bi, (big_s, big_d, nch) in enumerate(big):
            cw = big_s.shape[1] // nch
            for j in range(nch):
                eng = engs[(bi + j) % 3]
                eng.dma_start(big_s[:, j * cw:(j + 1) * cw],
                              big_d[:, j * cw:(j + 1) * cw])
        nc.sync.dma_start(A2_s[:], A2_d[:])
        nc.scalar.dma_start(r2_s[:], r2_d[:])

        # ping-pong lhsT buffers for the layer-2 root matmul: rows 0:64 get
        # h1^T via transpose-DMA each window, row 64 stays all-ones.
        h1T = [consts.tile([CIN2 + 1, P], dt.bfloat16, name=f"h1T{i}",
                           tag=f"h1T{i}")
               for i in range(2)]
        for hT in h1T:
            nc.vector.tensor_scalar(
                out=hT[CIN2:CIN2 + 1, :], in0=iota_s[0:1, :], scalar1=-1.0,
                scalar2=None, op0=mybir.AluOpType.is_ge)

        # PE warmup: a dense burst of back-to-back matmuls (~4us) flips the
        # HAM clock gate to 8/8 before layer 1 starts; with steady matmul
        # traffic afterwards the PE stays at 2.4 GHz instead of 1.2.
        if WARM_MM:
            wt = pp.tile([P, 1024], dt.float32, name="wt", tag="pu",
                         padded_shape=[P, 1024])
            for _ in range(WARM_MM):
                nc.tensor.matmul(wt[:, 0:512], lhsT=iota_s[0:32, :],
                                 rhs=xsg_s[0:32, 0:512], start=True,
                                 stop=True)

        def layer(is_l1, after_window=None):
            cin = CIN1 if is_l1 else CIN2
            cols = cin * COUT           # 512 or 4096
            nunits = max(1, cols // 1024)
            ucols = min(cols, 1024)
            uich = ucols // COUT        # 8 (L1) or 16 (L2)

            aggws = {}
            pend_q = []
            lag = L1_LAG if is_l1 else SCAT_LAG
            xgs = {}

            def prefetch_gather(g):
                if g >= NT or g in xgs:
                    return
                xg = xgp.tile([P, cin], dt.bfloat16, name="xg", tag="xg2",
                              bufs=GPRE + 4)
                nc.gpsimd.indirect_dma_start(
                    out=xg[:], out_offset=None, in_=hglob[:],
                    in_offset=IndirectOffsetOnAxis(ap=srcw_s[:, g:g + 1],
                                                   axis=0))
                xgs[g] = xg

            def emit_scat(p):
                # p: (w, last, items); items: ("fat", oh, q_bf16) |
                # ("dr", oh_fp8_pair, [qAB fp8 pair tiles])
                w, last, items = p
                aggw = aggws[w]
                nmm = 0
                for it in items:
                    if it[0] == "fat":
                        nmm += max(1, it[2].shape[1] // 512)
                    else:
                        nmm += len(it[2]) * (it[2][0].shape[1] // (2 * COUT)) // 8
                k = 0
                for it in items:
                    if it[0] == "fat":
                        # q holds (i,o)-ordered bf16, contiguous rhs
                        oh, qt = it[1], it[2]
                        for h in range(max(1, qt.shape[1] // 512)):
                            k += 1
                            hi = min(qt.shape[1], 512) // COUT
                            q3 = qt[:, h * 512:h * 512 + hi * COUT].rearrange(
                                "p (i o) -> p i o", i=hi)
                            nc.tensor.matmul(
                                aggw[:].unsqueeze(1).broadcast_to([P, hi, COUT]),
                                lhsT=oh[:], rhs=q3,
                                start=False, stop=(last and k == nmm),
                                skip_group_check=True)
                    else:  # "dr": fp8 DoubleRow over a tile pair
                        ohAB, qABs = it[1], it[2]
                        oh3 = ohAB[:].rearrange("p (c v) -> p c v", c=2)
                        for qt in qABs:
                            ich = qt.shape[1] // (2 * COUT)
                            q4 = qt[:].rearrange("p (c i o) -> p c i o", c=2,
                                                 i=ich)
                            for h in range(ich // 8):
                                k += 1
                                nc.tensor.matmul(
                                    aggw[:].unsqueeze(1).broadcast_to(
                                        [P, 8, COUT]),
                                    lhsT=oh3,
                                    rhs=q4[:, :, h * 8:(h + 1) * 8, :],
                                    start=False, stop=(last and k == nmm),
                                    skip_group_check=True,
                                    perf_mode=mybir.MatmulPerfMode.DoubleRow)
                if last:
                    # finalize window: copy PSUM -> SBUF, write out
                    if is_l1:
                        hw_ = outp.tile([P, COUT], dt.bfloat16, tag="h1w")
                        nc.scalar.copy(hw_[:], aggw[:])
                        nc.sync.dma_start(hloc[w * P:(w + 1) * P, :], hw_[:])
                    else:
                        ow = outp.tile([P, COUT], dt.float32, tag="outw")
                        nc.scalar.copy(ow[:], aggw[:])
                        nc.sync.dma_start(out_d[w * P:(w + 1) * P, :], ow[:])
                    if after_window is not None:
                        after_window(w)

            def gen_mms(pu, g, A_s, u):
                for h in range(ucols // 512):
                    c0 = u * ucols + h * 512
                    # spread the K=3 matmuls over the 4 PE row groups
                    rg = 32 * ((u * (ucols // 512) + h) % 4) if cols >= 2048 \
                        else 32 * (g % 4)
                    nc.tensor.matmul(
                        pu[:, h * 512:(h + 1) * 512],
                        lhsT=attr_s[rg:rg + 3, g * P:(g + 1) * P],
                        rhs=A_s[rg:rg + 3, c0:c0 + 512],
                        start=True, stop=True, tile_position=(rg, 0))

            def open_window(w):
                aggw = aggp.tile([P, COUT], dt.float32, tag="aggw")
                aggws[w] = aggw
                # root matmul (with bias folded in) opens the accumulation
                if is_l1:
                    nc.tensor.matmul(aggw[:],
                                     lhsT=xT9_s[:, w * P:(w + 1) * P],
                                     rhs=r1_s[:], start=True, stop=False)
                else:
                    # the transpose-DMA for this window was prefetched a
                    # window ahead so the root matmul never head-of-line
                    # blocks the PE queue waiting on DMA latency
                    nc.tensor.matmul(aggw[:], lhsT=h1T[w % 2][:], rhs=r2_s[:],
                                     start=True, stop=False)
                    if w + 1 < WPC:
                        nc.sync.dma_start_transpose(
                            h1T[(w + 1) % 2][0:CIN2, :],
                            hloc[(w + 1) * P:(w + 2) * P, :])

            if is_l1:
                # L1: tile PAIRS share one [P,1024] PSUM tile; one fused
                # relu+mult stt evacuates the pair to fp8 and a DoubleRow
                # matmul scatters both tiles' 256 edges in a single pass.
                for j in range(NT // 2):
                    g0 = 2 * j
                    w, t0 = divmod(g0, T)
                    while len(pend_q) > lag:
                        emit_scat(pend_q.pop(0))
                    if t0 == 0:
                        open_window(w)
                    pu = pp.tile([P, 1024], dt.float32, name="pu", tag="pu",
                                 padded_shape=[P, 1024])
                    gen_mms(pu[:, 0:512], g0, A1_s, 0)
                    gen_mms(pu[:, 512:1024], g0 + 1, A1_s, 0)
                    qt = qp.tile([P, 1024], dt.float8e4, name="qt1",
                                 tag="q1p", bufs=8)
                    xg_sl = xsg_s[:, g0 * cin:(g0 + 2) * cin]
                    nc.vector.scalar_tensor_tensor(
                        out=qt[:].rearrange("p (t o) -> p t o", t=16),
                        in0=pu[:].rearrange("p (t o) -> p t o", t=16),
                        scalar=0.0,
                        in1=xg_sl.to_broadcast([P, 2 * cin, COUT]),
                        op0=mybir.AluOpType.max, op1=mybir.AluOpType.mult)
                    items = [("dr", oh2_s[:, g0 * P:(g0 + 2) * P], [qt])]
                    pend_q.append((w, t0 + 1 == T - 1, items))
            else:
                nc.sync.dma_start_transpose(h1T[0][0:CIN2, :], hloc[0:P, :])
                state_pair = [None]
                for g in range(NT):
                    w, t = divmod(g, T)
                    while len(pend_q) > lag:
                        emit_scat(pend_q.pop(0))
                    if g == 0:
                        for gg in range(GPRE):
                            prefetch_gather(gg)
                    prefetch_gather(g + GPRE)
                    if t == 0:
                        open_window(w)
                    oh = oh1_s[:, g * P:(g + 1) * P]
                    xg = xgs.pop(g)
                    # x duplicated into adjacent pairs (for the 2x multiply)
                    xg2 = xgp.tile([P, 2 * cin], dt.bfloat16, name="xg2",
                                   tag="xg2d", bufs=6)
                    nc.gpsimd.tensor_copy(
                        xg2[:].rearrange("p (i c) -> p i c", c=2),
                        xg[:].unsqueeze(2).broadcast_to([P, cin, 2]))
                    half = g % 2
                    pi = g // 2
                    u_dr = 0 if pi < PREWORK_PAIRS else DR_CYCLE[pi % 4]
                    if half == 0:
                        ohAB = oh2_s[:, g * P:(g + 2) * P]
                        qABs = [qp.tile([P, 2 * 1024], dt.float8e4,
                                        name=f"qAB{u}", tag=f"qAB{u}",
                                        bufs=4 if u == 0 else 2)
                                for u in range(u_dr)]
                        state_pair[0] = (ohAB, qABs)
                    ohAB, qABs = state_pair[0]

                    items = []
                    for u in range(u_dr):
                        pu = pp.tile([P, ucols], dt.float32, name="pu",
                                     tag="pu", padded_shape=[P, 1024])
                        gen_mms(pu, g, A2_s, u)
                        # dr: fused relu+mult into the fp8 pair buffer
                        qh = qABs[u][:, half * 1024:(half + 1) * 1024]
                        xg_sl = xg[:, u * uich:(u + 1) * uich]
                        nc.vector.scalar_tensor_tensor(
                            out=qh.rearrange("p (i o) -> p i o", i=uich),
                            in0=pu[:].rearrange("p (i o) -> p i o", i=uich),
                            scalar=0.0,
                            in1=xg_sl.to_broadcast([P, uich, COUT]),
                            op0=mybir.AluOpType.max,
                            op1=mybir.AluOpType.mult)
                    # sv2 units, paired where possible: two ScalarE relus
                    # share one [P,2048] buffer so a single DVE 2x multiply
                    # covers both (halves the per-op overhead)
                    sv = list(range(u_dr, nunits))
                    vi = 0
                    while vi < len(sv):
                        grp = sv[vi:vi + 2]
                        vi += len(grp)
                        gw = len(grp) * ucols
                        ru = rup.tile([P, gw], dt.bfloat16, name="ru",
                                      tag=f"ru{len(grp)}", bufs=8)
                        for k, u in enumerate(grp):
                            pu = pp.tile([P, ucols], dt.float32, name="pu",
                                         tag="pu", padded_shape=[P, 1024])
                            gen_mms(pu, g, A2_s, u)
                            nc.scalar.activation(
                                out=ru[:, k * ucols:(k + 1) * ucols],
                                in_=pu[:],
                                func=mybir.ActivationFunctionType.Relu)
                        qt = qp.tile([P, gw], dt.bfloat16, name="qtS",
                                     tag=f"qS{len(grp)}", bufs=8)
                        u0 = grp[0]
                        gi = len(grp) * uich
                        x2 = xg2[:, u0 * 2 * uich:u0 * 2 * uich + 2 * gi]
                        nc.vector.tensor_tensor(
                            out=qt[:].rearrange("p (i o c) -> p i o c",
                                                i=gi, c=2),
                            in0=ru[:].rearrange("p (i o c) -> p i o c",
                                                i=gi, c=2),
                            in1=x2.rearrange("p (i c) -> p i c", c=2)
                                .unsqueeze(2)
                                .broadcast_to([P, gi, COUT // 2, 2]),
                            op=mybir.AluOpType.mult)
                        items.append(("fat", oh, qt))

                    if half == 1 and qABs:
                        items.append(("dr", ohAB, qABs))
                    pend_q.append((w, t == T - 1, items))
            for p in pend_q:
                emit_scat(p)

        # h1 is exchanged in chunks so the early AllGathers overlap layer-1
        # compute and the tail chunks are small.  hglob layout is chunk-major:
        # chunk j holds local rows [AGB[j]*128, AGB[j+1]*128) of every core,
        # cores stacked.  Gather indices are remapped on the host to match.
        def emit_ag(j):
            r0, r1 = AGB[j] * P, AGB[j + 1] * P
            nc.gpsimd.collective_compute(
                "AllGather", mybir.AluOpType.bypass,
                replica_groups=[list(range(NCORES))],
                ins=[hloc[r0:r1, :].opt()],
                outs=[hglob[r0 * NCORES:r1 * NCORES, :].opt()])

        def ag_after_window(w):
            if (w + 1) in AGB:
                emit_ag(AGB.index(w + 1) - 1)

        layer(True, after_window=ag_after_window)
        layer(False)

    nc.compile()
    return nc


def _pack(edge_index):
    """Relabel nodes into 128 windows of 128 nodes / exactly U edges each.

    Returns (perm, U, order) where perm[orig_node] = new node id and
    order = edge permutation grouping edges by destination window, padded.
    """
    dst = np.asarray(edge_index[1], dtype=np.int64)
    deg = np.bincount(dst, minlength=N).astype(np.int64)
    # LPT greedy: descending degree, least-loaded window with free slots
    nodes = np.argsort(-deg, kind="stable")
    loads = np.zeros(WINDOWS, dtype=np.int64)
    slots = np.zeros(WINDOWS, dtype=np.int64)
    wof = np.empty(N, dtype=np.int64)  # window of node
    for v in nodes:
        open_w = np.flatnonzero(slots < P)
        w = open_w[np.argmin(loads[open_w])]
        wof[v] = w
        loads[w] += deg[v]
        slots[w] += 1
    # repair toward exact target load by swapping nodes between windows
    target = E // WINDOWS
    if loads.max() > target:
        by_wd = {}  # (window, degree) -> list of nodes
        for v in range(N):
            by_wd.setdefault((wof[v], deg[v]), []).append(v)
        for _ in range(100000):
            over = int(np.argmax(loads))
            under = int(np.argmin(loads))
            if loads[over] <= target:
                break
            delta = min(loads[over] - target, target - loads[under])
            # find a swap pair with degree difference = d, largest d first
            done = False
            for d in range(int(delta), 0, -1):
                for da in range(int(deg.max()), d - 1, -1):
                    la = by_wd.get((over, da))
                    lb = by_wd.get((under, da - d))
                    if la and lb:
                        a, b = la.pop(), lb.pop()
                        wof[a], wof[b] = under, over
                        by_wd.setdefault((under, da), []).append(a)
                        by_wd.setdefault((over, da - d), []).append(b)
                        loads[over] -= d
                        loads[under] += d
                        done = True
                        break
                if done:
                    break
            if not done:
                break
    U = int(np.ceil(loads.max() / P) * P)
    # perm: nodes sorted by window -> new ids
    new_order = np.argsort(wof * N + np.arange(N), kind="stable")
    perm = np.empty(N, dtype=np.int64)
    perm[new_order] = np.arange(N)
    # edge order: group by destination window, pad each window to U
    ew = wof[dst]
    eorder = np.argsort(ew, kind="stable")
    counts = np.bincount(ew, minlength=WINDOWS)
    padded = np.full(WINDOWS * U, -1, dtype=np.int64)
    pos = 0
    for w in range(WINDOWS):
        c = int(counts[w])
        padded[w * U:w * U + c] = eorder[pos:pos + c]
        pos += c
    return perm, U, padded


def kernel(x, edge_index, edge_attr, A1, b1, A2, b2, root1, bias1, root2, bias2):
    x = np.asarray(x, dtype=np.float32)
    edge_index = np.asarray(edge_index)
    edge_attr = np.asarray(edge_attr, dtype=np.float32)

    perm, U, padded = _pack(edge_index)
    T = U // P
    NT = WPC * T
    key = U
    if key not in _cached:
        _cached[key] = _build_program(U)
    nc = _cached[key]

    src = np.asarray(edge_index[0], dtype=np.int64)
    dst = np.asarray(edge_index[1], dtype=np.int64)
    valid = padded >= 0
    pe = np.where(valid, padded, 0)
    # per padded-edge data
    a01 = edge_attr[pe]                      # [W*U, 2]
    aug = valid.astype(np.float32)
    attr3 = np.stack([a01[:, 0] * aug, a01[:, 1] * aug, aug]).astype(BF16)
    attrT_all = np.zeros((99, attr3.shape[1]), dtype=BF16)
    for rg in range(4):
        attrT_all[32 * rg:32 * rg + 3] = attr3
    # gather-index remap to the chunk-major hglob layout produced by the
    # split AllGather (chunk boundaries AGB, in windows per core)
    nn = np.arange(N, dtype=np.int64)
    cc_ = nn // NPC
    qq = nn % NPC
    agb = np.asarray(AGB, dtype=np.int64)
    jj = np.searchsorted(agb, qq // P, side="right") - 1
    base = agb[jj] * P
    csz = (agb[jj + 1] - agb[jj]) * P
    idxmap = base * NCORES + cc_ * csz + (qq - base)
    srcn_all = np.where(valid, idxmap[perm[src[pe]]], 0).astype(np.int32)
    dstn = perm[dst[pe]]
    wof_e = np.arange(WINDOWS).repeat(U)
    dstrel_all = np.where(valid, dstn - wof_e * P, 0).astype(np.float32)

    x_pi = np.empty_like(x)
    x_pi[perm] = x
    x_bf = x_pi.astype(BF16)
    # xbf rows live at remapped positions so one srcw feeds both layers
    x_bf_remap = np.empty_like(x_bf)
    x_bf_remap[idxmap] = x_bf

    def rep4(Aaug3):
        out = np.zeros((99, Aaug3.shape[1]), dtype=BF16)
        for rg in range(4):
            out[32 * rg:32 * rg + 3] = Aaug3
        return out
    A1aug3 = np.concatenate([A1, b1[None, :]], axis=0).astype(BF16)
    A2aug3 = np.concatenate([A2, b2[None, :]], axis=0).astype(BF16)
    r1aug = np.concatenate([root1, bias1[None, :]], axis=0).astype(BF16)
    r2aug = np.concatenate([root2, bias2[None, :]], axis=0).astype(BF16)
    iota_np = np.broadcast_to(np.arange(P, dtype=np.float32), (P, P)).astype(BF16)
    shared = {
        "A1aug": rep4(A1aug3), "A2aug": rep4(A2aug3),
        "r1aug": np.asarray(r1aug), "r2aug": np.asarray(r2aug),
        "iota": np.asarray(iota_np),
    }
    EPC = WPC * U
    in_maps = []
    ones9 = np.ones((1, NPC), dtype=BF16)
    vrange = np.arange(P, dtype=np.float32)
    for c in range(NCORES):
        s = c * EPC
        m = dict(shared)
        m["attrT"] = attrT_all[:, s:s + EPC].copy()
        # [P, NT] with column (w*T + t) = edges [wU + t*128 : wU + (t+1)*128)
        srcw = np.ascontiguousarray(srcn_all[s:s + EPC].reshape(NT, P).T)
        m["srcw"] = srcw
        dstw = dstrel_all[s:s + EPC].reshape(NT, P).T  # [P, NT]
        # host-precomputed layer-1 gathered features and one-hot matrices
        m["xsg"] = x_bf_remap[srcw].reshape(P, NT * CIN1)
        m["xsg2"] = np.repeat(m["xsg"], 2, axis=1)
        oh = (dstw[:, :, None] == vrange[None, None, :])  # [P, NT, 128]
        m["oh1"] = oh.astype(BF16).reshape(P, NT * P)
        m["oh2"] = oh.astype(FP8).reshape(P, NT * P)
        xTc = np.ascontiguousarray(x_bf[c * NPC:(c + 1) * NPC].T)
        m["xT9"] = np.concatenate([xTc, ones9], axis=0)
        in_maps.append(m)

    res = run_bass_kernel_spmd(nc, in_maps, list(range(NCORES)),
                               **kernel.run_kwargs)
    kernel.last_result = res
    out_pi = np.concatenate([res.results[c]["out"] for c in range(NCORES)], axis=0)
    return out_pi[perm]


kernel.run_kwargs = {}
kernel.last_result = None
